# revision 4
# baseline (speedup 1.0000x reference)
"""3-layer GAT (N=20000, E=400000, 768 -> 4x128 -> 4x128 -> 128) on 8 TRN2
NeuronCores.

Sharding: nodes are range-partitioned across the 8 cores (2500 dst nodes
each); every edge lives on the core that owns its dst node, so the
segment-softmax aggregation is core-local.  Between layers the full node
feature matrix (the gather source) is re-assembled on the host and
re-broadcast - the halo exchange of the sharding hint, done host-side so
each device phase is a self-contained SPMD NEFF.

Device phases (all 8 cores, SPMD):
  K1: h1 = x_shard @ W1 (bf16 matmul), per-head attention dots.
  K2: gather h1[src] rows (dma_gather), one-hot x weight matmul-scatter into
      PSUM per 128-dst block, +b1, ELU, transpose, @W2, attention dots.
  K3: same as K2 with W3 (512->128, layer-3 head dots).
  K4: gather h3[src], 1-head weighted scatter, +b3 -> final f32 output.

The host computes per-edge softmax weights w' = exp(leakyrelu(.))/s between
phases (0.02% of FLOPs) and folds the softmax denominator into w', so the
device aggregation is a plain weighted segment-sum done as 128x128 one-hot
matmuls on the tensor engine.
"""

import numpy as np
import ml_dtypes

# ---------------------------------------------------------------- constants
N = 20000
E = 400000
IN_C = 768
HID = 128
HEADS = 4
NEG_SLOPE = 0.2
NCORES = 8
NPC = N // NCORES          # nodes per core (2500)
NPAD = 2560                # padded to 20 blocks of 128
NBLK = NPAD // 128         # dst blocks per core
ETOT = E + N               # edges incl self-loops

BF16 = ml_dtypes.bfloat16

TRACE = False              # test.py flips this to collect HW exec times
LAST_EXEC_NS = {}          # phase -> max-over-cores exec ns (when TRACE)

_PROG_CACHE = {}


# ------------------------------------------------------------- host helpers
def _leaky_exp(e):
    e = np.where(e > 0.0, e, NEG_SLOPE * e)
    return np.exp(e, dtype=np.float64)


def _edge_weights(alpha, src_all, dst_all, nheads):
    """alpha: [N, 2*nheads] f32 (src dots | dst dots) -> normalized softmax
    weights [ETOT, nheads] f32, replicating the reference's numerics exactly:
    in this environment jax.ops.segment_max evaluates as a segment *sum*, so
    the "max" subtracted before exp is the per-(dst, head) sum of logits, and
    fully-underflowed segments yield all-zero weights via the +1e-16 guard."""
    asrc = alpha[:, :nheads].astype(np.float32)
    adst = alpha[:, nheads:].astype(np.float32)
    e = asrc[src_all] + adst[dst_all]
    e = np.where(e > 0.0, e, np.float32(NEG_SLOPE) * e).astype(np.float32)
    m = np.zeros((N, nheads), np.float32)
    np.add.at(m, dst_all, e)
    ee = np.exp(e - m[dst_all], dtype=np.float32)
    s = np.zeros((N, nheads), np.float32)
    np.add.at(s, dst_all, ee)
    wp = ee / (s[dst_all] + np.float32(1e-16))
    return wp.astype(np.float32)


def _wrap128(a):
    """[CHT*128, ...] -> [128, CHT, ...]"""
    cht = a.shape[0] // 128
    return np.ascontiguousarray(np.swapaxes(a.reshape(cht, 128, *a.shape[1:]), 0, 1))


def _bcast(v, width):
    return np.ascontiguousarray(np.broadcast_to(np.asarray(v, np.float32).reshape(1, width), (128, width)))


class _Graph:
    """Static per-core edge layout shared by phases K2-K4."""

    def __init__(self, edge_index):
        src_all = np.concatenate([edge_index[0], np.arange(N)]).astype(np.int64)
        dst_all = np.concatenate([edge_index[1], np.arange(N)]).astype(np.int64)
        self.src_all, self.dst_all = src_all, dst_all

        core = dst_all // NPC
        dloc_all = dst_all - core * NPC
        blk_all = dloc_all // 128

        # per (core, block) edge-id lists
        order = np.lexsort((blk_all, core))                 # sort by core, then block
        e_sorted = order
        c_sorted = core[order]
        b_sorted = blk_all[order]
        # boundaries via searchsorted on combined key
        key = c_sorted * NBLK + b_sorted
        starts = np.searchsorted(key, np.arange(NCORES * NBLK))
        ends = np.searchsorted(key, np.arange(NCORES * NBLK) + 1)
        counts = (ends - starts).reshape(NCORES, NBLK)

        self.cpb = np.maximum(1, -(-counts.max(axis=0) // 128))   # chunks per block
        self.cht = int(self.cpb.sum())
        self.off = np.concatenate([[0], np.cumsum(self.cpb)])      # chunk offsets

        epad = self.cht * 128
        self.sel = np.full((NCORES, epad), -1, np.int64)
        for k in range(NCORES):
            for b in range(NBLK):
                s, e = starts[k * NBLK + b], ends[k * NBLK + b]
                o = self.off[b] * 128
                self.sel[k, o:o + (e - s)] = e_sorted[s:e]

        # per-core static inputs
        self.gidx = []
        self.dloc = []
        for k in range(NCORES):
            sel = self.sel[k]
            valid = sel >= 0
            g = np.where(valid, src_all[np.maximum(sel, 0)], 0)
            d = np.where(valid, dloc_all[np.maximum(sel, 0)] % 128, 0)
            self.gidx.append(_wrap128(g.astype(np.int32)))
            self.dloc.append(_wrap128(d.astype(np.float32)))

    def wq(self, wp, nheads):
        """per-core wrapped w' arrays [128, CHT, nheads] f32"""
        out = []
        for k in range(NCORES):
            sel = self.sel[k]
            w = np.where((sel >= 0)[:, None], wp[np.maximum(sel, 0)], 0.0)
            out.append(_wrap128(np.ascontiguousarray(w, np.float32)))
        return out


# ------------------------------------------------------------ bass programs
def _mk_nc():
    import concourse.bacc as bacc
    return bacc.Bacc("TRN2", target_bir_lowering=False, debug=False,
                     num_devices=NCORES)


def _build_k1():
    import concourse.mybir as mybir
    from concourse import tile
    nc = _mk_nc()
    dt = mybir.dt
    KC = IN_C // 128                                           # 6
    xT = nc.dram_tensor("xT", [KC, 128, NPAD], dt.bfloat16, kind="ExternalInput")
    W = nc.dram_tensor("W", [KC, 128, 512], dt.bfloat16, kind="ExternalInput")
    asb = nc.dram_tensor("asb", [128, 512], dt.float32, kind="ExternalInput")
    adb = nc.dram_tensor("adb", [128, 512], dt.float32, kind="ExternalInput")
    h_o = nc.dram_tensor("h", [NPAD, 512], dt.bfloat16, kind="ExternalOutput")
    al_o = nc.dram_tensor("al", [NPAD, 8], dt.float32, kind="ExternalOutput")

    with tile.TileContext(nc) as tc:
        with tc.tile_pool(name="pre", bufs=1) as pre, \
             tc.tile_pool(name="work", bufs=3) as work, \
             tc.tile_pool(name="ps", bufs=2, space="PSUM") as ps:
            xT_sb = pre.tile([128, KC, NPAD], dt.bfloat16)
            W_sb = pre.tile([128, KC, 512], dt.bfloat16)
            for i in range(KC):
                nc.sync.dma_start(xT_sb[:, i, :], xT[i])
                nc.sync.dma_start(W_sb[:, i, :], W[i])
            as_sb = pre.tile([128, 512], dt.float32)
            ad_sb = pre.tile([128, 512], dt.float32)
            nc.sync.dma_start(as_sb[:], asb[:])
            nc.sync.dma_start(ad_sb[:], adb[:])

            for nt in range(NBLK):
                acc = ps.tile([128, 512], dt.float32, tag="acc")
                for i in range(KC):
                    nc.tensor.matmul(acc[:], lhsT=xT_sb[:, i, nt * 128:(nt + 1) * 128],
                                     rhs=W_sb[:, i, :], start=(i == 0), stop=(i == KC - 1))
                hbf = work.tile([128, 512], dt.bfloat16, tag="hbf")
                nc.scalar.activation(hbf[:], acc[:], mybir.ActivationFunctionType.Copy)
                nc.sync.dma_start(h_o[nt * 128:(nt + 1) * 128, :], hbf[:])
                al = work.tile([128, 8], dt.float32, tag="al")
                tmp = work.tile([128, 512], dt.float32, tag="tmp")
                nc.vector.tensor_tensor(tmp[:], acc[:], as_sb[:], mybir.AluOpType.mult)
                nc.vector.tensor_reduce(al[:, 0:4], tmp.rearrange("p (h c) -> p h c", h=4),
                                        mybir.AxisListType.X, mybir.AluOpType.add)
                nc.vector.tensor_tensor(tmp[:], acc[:], ad_sb[:], mybir.AluOpType.mult)
                nc.vector.tensor_reduce(al[:, 4:8], tmp.rearrange("p (h c) -> p h c", h=4),
                                        mybir.AxisListType.X, mybir.AluOpType.add)
                nc.sync.dma_start(al_o[nt * 128:(nt + 1) * 128, :], al[:])
    nc.compile()
    return nc


def _build_edge(graph, fin, fout, n_alpha_heads, with_mm):
    """K2/K3 (with_mm=True): gather fin-wide rows, 4-head weighted scatter,
    +bias, ELU, transpose, @W -> fout, alpha dots.
    K4 (with_mm=False): 1-head scatter over fin(=128)-wide rows, +bias, f32 out.
    """
    import concourse.mybir as mybir
    from concourse import tile
    nc = _mk_nc()
    dt = mybir.dt
    cpb, cht, off = graph.cpb, graph.cht, graph.off
    nh = HEADS if with_mm else 1

    h_ext = nc.dram_tensor("h_ext", [N, fin // 2], dt.float32, kind="ExternalInput")
    gidx = nc.dram_tensor("gidx", [128, cht], dt.int32, kind="ExternalInput")
    dloc = nc.dram_tensor("dloc", [128, cht], dt.float32, kind="ExternalInput")
    wq = nc.dram_tensor("wq", [128, cht, nh], dt.float32, kind="ExternalInput")
    bias = nc.dram_tensor("bias", [128, fin], dt.float32, kind="ExternalInput")
    if with_mm:
        W = nc.dram_tensor("W", [4, 128, fout], dt.bfloat16, kind="ExternalInput")
        asb = nc.dram_tensor("asb", [128, fout], dt.float32, kind="ExternalInput")
        adb = nc.dram_tensor("adb", [128, fout], dt.float32, kind="ExternalInput")
        ident = nc.dram_tensor("ident", [128, 128], dt.bfloat16, kind="ExternalInput")
        h_o = nc.dram_tensor("h", [NPAD, fout], dt.bfloat16, kind="ExternalOutput")
        al_o = nc.dram_tensor("al", [NPAD, 2 * n_alpha_heads], dt.float32,
                              kind="ExternalOutput")
    else:
        y_o = nc.dram_tensor("y", [NPAD, fin], dt.float32, kind="ExternalOutput")

    with tile.TileContext(nc) as tc:
        with tc.tile_pool(name="pre", bufs=1) as pre, \
             tc.tile_pool(name="fpool", bufs=2) as fpool, \
             tc.tile_pool(name="work", bufs=3) as work, \
             tc.tile_pool(name="ps", bufs=2, space="PSUM") as ps, \
             tc.tile_pool(name="pst", bufs=2, space="PSUM") as pst:
            gidx_sb = pre.tile([128, cht], dt.int32)
            nc.sync.dma_start(gidx_sb[:], gidx[:])
            dloc_sb = pre.tile([128, cht], dt.float32)
            nc.sync.dma_start(dloc_sb[:], dloc[:])
            wq_sb = pre.tile([128, cht, nh], dt.float32)
            nc.sync.dma_start(wq_sb[:], wq[:])
            bias_sb = pre.tile([128, fin], dt.float32)
            nc.sync.dma_start(bias_sb[:], bias[:])
            if with_mm:
                W_sb = pre.tile([128, 4, fout], dt.bfloat16)
                for i in range(4):
                    nc.sync.dma_start(W_sb[:, i, :], W[i])
                as_sb = pre.tile([128, fout], dt.float32)
                ad_sb = pre.tile([128, fout], dt.float32)
                nc.sync.dma_start(as_sb[:], asb[:])
                nc.sync.dma_start(ad_sb[:], adb[:])
                id_sb = pre.tile([128, 128], dt.bfloat16)
                nc.sync.dma_start(id_sb[:], ident[:])

            iota_i = pre.tile([128, 128], dt.int32)
            nc.gpsimd.iota(iota_i[:], pattern=[[1, 128]], base=0, channel_multiplier=0)
            iota_f = pre.tile([128, 128], dt.float32)
            nc.vector.tensor_copy(iota_f[:], iota_i[:])

            for b in range(NBLK):
                nchunk = int(cpb[b])
                c0 = int(off[b])
                F = fpool.tile([128, nchunk, fin // 2], dt.float32, tag="F")
                import concourse.bass as bass
                for j in range(nchunk):
                    nc.gpsimd.indirect_dma_start(
                        out=F[:, j, :], out_offset=None, in_=h_ext[:],
                        in_offset=bass.IndirectOffsetOnAxis(
                            ap=gidx_sb[:, c0 + j:c0 + j + 1], axis=0))
                agg = ps.tile([128, fin], dt.float32, tag="agg")
                for j in range(nchunk):
                    c = c0 + j
                    Fb = F[:, j, :].bitcast(dt.bfloat16)
                    if with_mm:
                        match = work.tile([128, 128], dt.bfloat16, tag="match")
                        nc.vector.tensor_scalar(
                            out=match[:], in0=iota_f[:], scalar1=dloc_sb[:, c:c + 1],
                            scalar2=None, op0=mybir.AluOpType.is_equal)
                        Fw = work.tile([128, fin], dt.bfloat16, tag="Fw")
                        for h in range(4):
                            nc.vector.tensor_scalar(
                                out=Fw[:, h * 128:(h + 1) * 128],
                                in0=Fb[:, h * 128:(h + 1) * 128],
                                scalar1=wq_sb[:, c, h:h + 1], scalar2=None,
                                op0=mybir.AluOpType.mult)
                        rhs = Fw[:]
                        lhsT = match[:]
                    else:
                        mw = work.tile([128, 128], dt.bfloat16, tag="mw")
                        nc.vector.tensor_scalar(
                            out=mw[:], in0=iota_f[:], scalar1=dloc_sb[:, c:c + 1],
                            scalar2=wq_sb[:, c, 0:1], op0=mybir.AluOpType.is_equal,
                            op1=mybir.AluOpType.mult)
                        rhs = Fb
                        lhsT = mw[:]
                    nc.tensor.matmul(agg[:], lhsT=lhsT, rhs=rhs,
                                     start=(j == 0), stop=(j == nchunk - 1))

                if not with_mm:
                    y_sb = work.tile([128, fin], dt.float32, tag="ysb")
                    nc.vector.tensor_tensor(y_sb[:], agg[:], bias_sb[:], mybir.AluOpType.add)
                    nc.sync.dma_start(y_o[b * 128:(b + 1) * 128, :], y_sb[:])
                    continue

                # out1 = ELU(agg + bias)  (f32), stored bf16 for transpose
                t0 = work.tile([128, fin], dt.float32, tag="t0")
                nc.vector.tensor_tensor(t0[:], agg[:], bias_sb[:], mybir.AluOpType.add)
                m = work.tile([128, fin], dt.float32, tag="m")
                nc.vector.tensor_scalar(out=m[:], in0=t0[:], scalar1=0.0, scalar2=None,
                                        op0=mybir.AluOpType.min)
                ex = work.tile([128, fin], dt.float32, tag="ex")
                nc.scalar.activation(ex[:], m[:], mybir.ActivationFunctionType.Exp)
                r = work.tile([128, fin], dt.float32, tag="r")
                nc.vector.tensor_scalar(out=r[:], in0=t0[:], scalar1=0.0, scalar2=-1.0,
                                        op0=mybir.AluOpType.max, op1=mybir.AluOpType.add)
                o1 = work.tile([128, fin], dt.bfloat16, tag="o1")
                nc.vector.tensor_tensor(o1[:], ex[:], r[:], mybir.AluOpType.add)

                # transpose per 128-col slice -> [c, d] tiles
                pt = pst.tile([128, fin], dt.bfloat16, tag="pt")
                for h in range(4):
                    nc.tensor.transpose(pt[:, h * 128:(h + 1) * 128],
                                        o1[:, h * 128:(h + 1) * 128], id_sb[:])
                o1T = work.tile([128, fin], dt.bfloat16, tag="o1T")
                nc.vector.tensor_copy(o1T[:], pt[:])

                hp = ps.tile([128, fout], dt.float32, tag="hp")
                for i in range(4):
                    nc.tensor.matmul(hp[:], lhsT=o1T[:, i * 128:(i + 1) * 128],
                                     rhs=W_sb[:, i, :], start=(i == 0), stop=(i == 3))
                hbf = work.tile([128, fout], dt.bfloat16, tag="hbf")
                nc.scalar.activation(hbf[:], hp[:], mybir.ActivationFunctionType.Copy)
                nc.sync.dma_start(h_o[b * 128:(b + 1) * 128, :], hbf[:])

                nah = n_alpha_heads
                al = work.tile([128, 2 * nah], dt.float32, tag="al")
                tmp = work.tile([128, fout], dt.float32, tag="tmp")
                nc.vector.tensor_tensor(tmp[:], hp[:], as_sb[:], mybir.AluOpType.mult)
                nc.vector.tensor_reduce(al[:, 0:nah],
                                        tmp.rearrange("p (h c) -> p h c", h=nah),
                                        mybir.AxisListType.X, mybir.AluOpType.add)
                nc.vector.tensor_tensor(tmp[:], hp[:], ad_sb[:], mybir.AluOpType.mult)
                nc.vector.tensor_reduce(al[:, nah:2 * nah],
                                        tmp.rearrange("p (h c) -> p h c", h=nah),
                                        mybir.AxisListType.X, mybir.AluOpType.add)
                nc.sync.dma_start(al_o[b * 128:(b + 1) * 128, :], al[:])
    nc.compile()
    return nc


def _get_progs(graph):
    key = tuple(graph.cpb.tolist())
    if key not in _PROG_CACHE:
        _PROG_CACHE[key] = {
            "k1": _build_k1(),
            "k2": _build_edge(graph, 512, 512, HEADS, True),
            "k3": _build_edge(graph, 512, HID, 1, True),
            "k4": _build_edge(graph, HID, 0, 0, False),
        }
    return _PROG_CACHE[key]


def _run(nc, in_maps, phase):
    from concourse.bass_utils import run_bass_kernel_spmd
    res = run_bass_kernel_spmd(nc, in_maps, list(range(NCORES)), trace=TRACE,
                               trace_cores=list(range(NCORES)) if TRACE else None)
    if TRACE:
        LAST_EXEC_NS[phase] = res.exec_time_ns
    return res.results


# ------------------------------------------------------------------- kernel
def kernel(x, edge_index, W1, a1_src, a1_dst, b1, W2, a2_src, a2_dst, b2,
           W3, a3_src, a3_dst, b3):
    x = np.asarray(x, np.float32)
    edge_index = np.asarray(edge_index)
    graph = _Graph(edge_index)
    progs = _get_progs(graph)
    src_all, dst_all = graph.src_all, graph.dst_all

    # ---- K1: h1 = x @ W1 ----
    W1r = np.ascontiguousarray(
        np.asarray(W1, np.float32).reshape(6, 128, 512)).astype(BF16)
    a1s_bc = _bcast(np.asarray(a1_src, np.float32).reshape(-1), 512)
    a1d_bc = _bcast(np.asarray(a1_dst, np.float32).reshape(-1), 512)
    in_maps = []
    for k in range(NCORES):
        xs = np.zeros((NPAD, IN_C), np.float32)
        xs[:NPC] = x[k * NPC:(k + 1) * NPC]
        xT = np.ascontiguousarray(xs.T.reshape(6, 128, NPAD)).astype(BF16)
        in_maps.append({"xT": xT, "W": W1r, "asb": a1s_bc, "adb": a1d_bc})
    res = _run(progs["k1"], in_maps, "k1")
    h1 = np.ascontiguousarray(
        np.concatenate([r["h"][:NPC] for r in res], axis=0))        # bf16 [N,512]
    al1 = np.concatenate([r["al"][:NPC] for r in res], axis=0)      # f32 [N,8]

    ident = np.eye(128, dtype=BF16)

    # ---- K2 ----
    wp1 = _edge_weights(al1, src_all, dst_all, HEADS)
    wq1 = graph.wq(wp1, HEADS)
    W2r = np.ascontiguousarray(
        np.asarray(W2, np.float32).reshape(4, 128, 512)).astype(BF16)
    a2s_bc = _bcast(np.asarray(a2_src, np.float32).reshape(-1), 512)
    a2d_bc = _bcast(np.asarray(a2_dst, np.float32).reshape(-1), 512)
    b1_bc = _bcast(np.asarray(b1, np.float32), 512)
    in_maps = [{"h_ext": h1.view(np.float32), "gidx": graph.gidx[k], "dloc": graph.dloc[k],
                "wq": wq1[k], "bias": b1_bc, "W": W2r, "asb": a2s_bc,
                "adb": a2d_bc, "ident": ident} for k in range(NCORES)]
    res = _run(progs["k2"], in_maps, "k2")
    h2 = np.ascontiguousarray(np.concatenate([r["h"][:NPC] for r in res], axis=0))
    al2 = np.concatenate([r["al"][:NPC] for r in res], axis=0)

    # ---- K3 ----
    wp2 = _edge_weights(al2, src_all, dst_all, HEADS)
    wq2 = graph.wq(wp2, HEADS)
    W3r = np.ascontiguousarray(
        np.asarray(W3, np.float32).reshape(4, 128, HID)).astype(BF16)
    a3s_bc = _bcast(np.asarray(a3_src, np.float32).reshape(-1), HID)
    a3d_bc = _bcast(np.asarray(a3_dst, np.float32).reshape(-1), HID)
    b2_bc = _bcast(np.asarray(b2, np.float32), 512)
    in_maps = [{"h_ext": h2.view(np.float32), "gidx": graph.gidx[k], "dloc": graph.dloc[k],
                "wq": wq2[k], "bias": b2_bc, "W": W3r, "asb": a3s_bc,
                "adb": a3d_bc, "ident": ident} for k in range(NCORES)]
    res = _run(progs["k3"], in_maps, "k3")
    h3 = np.ascontiguousarray(
        np.concatenate([r["h"][:NPC] for r in res], axis=0))        # [N,128] bf16
    al3 = np.concatenate([r["al"][:NPC] for r in res], axis=0)      # [N,2]

    # ---- K4 ----
    wp3 = _edge_weights(al3, src_all, dst_all, 1)
    wq3 = graph.wq(wp3, 1)
    b3_bc = _bcast(np.asarray(b3, np.float32), HID)
    in_maps = [{"h_ext": h3.view(np.float32), "gidx": graph.gidx[k], "dloc": graph.dloc[k],
                "wq": wq3[k], "bias": b3_bc} for k in range(NCORES)]
    res = _run(progs["k4"], in_maps, "k4")
    y = np.concatenate([r["y"][:NPC] for r in res], axis=0)
    return y.astype(np.float32)


# revision 5
# speedup vs baseline: 1.0012x; 1.0012x over previous
"""3-layer GAT (N=20000, E=400000, 768 -> 4x128 -> 4x128 -> 128) on 8 TRN2
NeuronCores.

Sharding: nodes are range-partitioned across the 8 cores (2500 dst nodes
each); every edge lives on the core that owns its dst node, so the
segment-softmax aggregation is core-local.  Between layers the full node
feature matrix (the gather source) is re-assembled on the host and
re-broadcast - the halo exchange of the sharding hint, done host-side so
each device phase is a self-contained SPMD NEFF.

Device phases (all 8 cores, SPMD):
  K1: h1 = x_shard @ W1 (bf16 matmul), per-head attention dots.
  K2: gather h1[src] rows (dma_gather), one-hot x weight matmul-scatter into
      PSUM per 128-dst block, +b1, ELU, transpose, @W2, attention dots.
  K3: same as K2 with W3 (512->128, layer-3 head dots).
  K4: gather h3[src], 1-head weighted scatter, +b3 -> final f32 output.

The host computes per-edge softmax weights w' = exp(leakyrelu(.))/s between
phases (0.02% of FLOPs) and folds the softmax denominator into w', so the
device aggregation is a plain weighted segment-sum done as 128x128 one-hot
matmuls on the tensor engine.
"""

import numpy as np
import ml_dtypes

# ---------------------------------------------------------------- constants
N = 20000
E = 400000
IN_C = 768
HID = 128
HEADS = 4
NEG_SLOPE = 0.2
NCORES = 8
NPC = N // NCORES          # nodes per core (2500)
NPAD = 2560                # padded to 20 blocks of 128
NBLK = NPAD // 128         # dst blocks per core
ETOT = E + N               # edges incl self-loops

BF16 = ml_dtypes.bfloat16

TRACE = False              # test.py flips this to collect HW exec times
LAST_EXEC_NS = {}          # phase -> max-over-cores exec ns (when TRACE)

_PROG_CACHE = {}


# ------------------------------------------------------------- host helpers
def _leaky_exp(e):
    e = np.where(e > 0.0, e, NEG_SLOPE * e)
    return np.exp(e, dtype=np.float64)


def _edge_weights(alpha, src_all, dst_all, nheads):
    """alpha: [N, 2*nheads] f32 (src dots | dst dots) -> normalized softmax
    weights [ETOT, nheads] f32, replicating the reference's numerics exactly:
    in this environment jax.ops.segment_max evaluates as a segment *sum*, so
    the "max" subtracted before exp is the per-(dst, head) sum of logits, and
    fully-underflowed segments yield all-zero weights via the +1e-16 guard."""
    asrc = alpha[:, :nheads].astype(np.float32)
    adst = alpha[:, nheads:].astype(np.float32)
    e = asrc[src_all] + adst[dst_all]
    e = np.where(e > 0.0, e, np.float32(NEG_SLOPE) * e).astype(np.float32)
    m = np.zeros((N, nheads), np.float32)
    np.add.at(m, dst_all, e)
    ee = np.exp(e - m[dst_all], dtype=np.float32)
    s = np.zeros((N, nheads), np.float32)
    np.add.at(s, dst_all, ee)
    wp = ee / (s[dst_all] + np.float32(1e-16))
    return wp.astype(np.float32)


def _wrap128(a):
    """[CHT*128, ...] -> [128, CHT, ...]"""
    cht = a.shape[0] // 128
    return np.ascontiguousarray(np.swapaxes(a.reshape(cht, 128, *a.shape[1:]), 0, 1))


def _bcast(v, width):
    return np.ascontiguousarray(np.broadcast_to(np.asarray(v, np.float32).reshape(1, width), (128, width)))


class _Graph:
    """Static per-core edge layout shared by phases K2-K4."""

    def __init__(self, edge_index):
        src_all = np.concatenate([edge_index[0], np.arange(N)]).astype(np.int64)
        dst_all = np.concatenate([edge_index[1], np.arange(N)]).astype(np.int64)
        self.src_all, self.dst_all = src_all, dst_all

        core = dst_all // NPC
        dloc_all = dst_all - core * NPC
        blk_all = dloc_all // 128

        # per (core, block) edge-id lists
        order = np.lexsort((blk_all, core))                 # sort by core, then block
        e_sorted = order
        c_sorted = core[order]
        b_sorted = blk_all[order]
        # boundaries via searchsorted on combined key
        key = c_sorted * NBLK + b_sorted
        starts = np.searchsorted(key, np.arange(NCORES * NBLK))
        ends = np.searchsorted(key, np.arange(NCORES * NBLK) + 1)
        counts = (ends - starts).reshape(NCORES, NBLK)

        self.cpb = np.maximum(1, -(-counts.max(axis=0) // 128))   # chunks per block
        self.cht = int(self.cpb.sum())
        self.off = np.concatenate([[0], np.cumsum(self.cpb)])      # chunk offsets

        epad = self.cht * 128
        self.sel = np.full((NCORES, epad), -1, np.int64)
        for k in range(NCORES):
            for b in range(NBLK):
                s, e = starts[k * NBLK + b], ends[k * NBLK + b]
                o = self.off[b] * 128
                self.sel[k, o:o + (e - s)] = e_sorted[s:e]

        # per-core static inputs
        self.gidx = []
        self.dloc = []
        for k in range(NCORES):
            sel = self.sel[k]
            valid = sel >= 0
            g = np.where(valid, src_all[np.maximum(sel, 0)], 0)
            d = np.where(valid, dloc_all[np.maximum(sel, 0)] % 128, 0)
            self.gidx.append(_wrap128(g.astype(np.int32)))
            self.dloc.append(_wrap128(d.astype(np.float32)))

    def wq(self, wp, nheads):
        """per-core wrapped w' arrays [128, CHT, nheads] f32"""
        out = []
        for k in range(NCORES):
            sel = self.sel[k]
            w = np.where((sel >= 0)[:, None], wp[np.maximum(sel, 0)], 0.0)
            out.append(_wrap128(np.ascontiguousarray(w, np.float32)))
        return out


# ------------------------------------------------------------ bass programs
def _mk_nc():
    import concourse.bacc as bacc
    return bacc.Bacc("TRN2", target_bir_lowering=False, debug=False,
                     num_devices=NCORES)


def _build_k1():
    import concourse.mybir as mybir
    from concourse import tile
    nc = _mk_nc()
    dt = mybir.dt
    KC = IN_C // 128                                           # 6
    xT = nc.dram_tensor("xT", [KC, 128, NPAD], dt.bfloat16, kind="ExternalInput")
    W = nc.dram_tensor("W", [KC, 128, 512], dt.bfloat16, kind="ExternalInput")
    asb = nc.dram_tensor("asb", [128, 512], dt.float32, kind="ExternalInput")
    adb = nc.dram_tensor("adb", [128, 512], dt.float32, kind="ExternalInput")
    h_o = nc.dram_tensor("h", [NPAD, 512], dt.bfloat16, kind="ExternalOutput")
    al_o = nc.dram_tensor("al", [NPAD, 8], dt.float32, kind="ExternalOutput")

    with tile.TileContext(nc) as tc:
        with tc.tile_pool(name="pre", bufs=1) as pre, \
             tc.tile_pool(name="work", bufs=3) as work, \
             tc.tile_pool(name="ps", bufs=2, space="PSUM") as ps:
            xT_sb = pre.tile([128, KC, NPAD], dt.bfloat16)
            W_sb = pre.tile([128, KC, 512], dt.bfloat16)
            for i in range(KC):
                nc.sync.dma_start(xT_sb[:, i, :], xT[i])
                nc.sync.dma_start(W_sb[:, i, :], W[i])
            as_sb = pre.tile([128, 512], dt.float32)
            ad_sb = pre.tile([128, 512], dt.float32)
            nc.sync.dma_start(as_sb[:], asb[:])
            nc.sync.dma_start(ad_sb[:], adb[:])

            for nt in range(NBLK):
                acc = ps.tile([128, 512], dt.float32, tag="acc")
                for i in range(KC):
                    nc.tensor.matmul(acc[:], lhsT=xT_sb[:, i, nt * 128:(nt + 1) * 128],
                                     rhs=W_sb[:, i, :], start=(i == 0), stop=(i == KC - 1))
                hbf = work.tile([128, 512], dt.bfloat16, tag="hbf")
                nc.scalar.activation(hbf[:], acc[:], mybir.ActivationFunctionType.Copy)
                nc.sync.dma_start(h_o[nt * 128:(nt + 1) * 128, :], hbf[:])
                al = work.tile([128, 8], dt.float32, tag="al")
                tmp = work.tile([128, 512], dt.float32, tag="tmp")
                nc.vector.tensor_tensor(tmp[:], acc[:], as_sb[:], mybir.AluOpType.mult)
                nc.vector.tensor_reduce(al[:, 0:4], tmp.rearrange("p (h c) -> p h c", h=4),
                                        mybir.AxisListType.X, mybir.AluOpType.add)
                nc.vector.tensor_tensor(tmp[:], acc[:], ad_sb[:], mybir.AluOpType.mult)
                nc.vector.tensor_reduce(al[:, 4:8], tmp.rearrange("p (h c) -> p h c", h=4),
                                        mybir.AxisListType.X, mybir.AluOpType.add)
                nc.sync.dma_start(al_o[nt * 128:(nt + 1) * 128, :], al[:])
    nc.compile()
    return nc


def _build_edge(graph, fin, fout, n_alpha_heads, with_mm):
    """K2/K3 (with_mm=True): gather fin-wide rows, 4-head weighted scatter,
    +bias, ELU, transpose, @W -> fout, alpha dots.
    K4 (with_mm=False): 1-head scatter over fin(=128)-wide rows, +bias, f32 out.
    """
    import concourse.bass as bass
    import concourse.mybir as mybir
    from concourse import tile
    nc = _mk_nc()
    dt = mybir.dt
    cpb, cht, off = graph.cpb, graph.cht, graph.off
    nh = HEADS if with_mm else 1

    h_ext = nc.dram_tensor("h_ext", [N, fin // 2], dt.float32, kind="ExternalInput")
    gidx = nc.dram_tensor("gidx", [128, cht], dt.int32, kind="ExternalInput")
    dloc = nc.dram_tensor("dloc", [128, cht], dt.float32, kind="ExternalInput")
    wq = nc.dram_tensor("wq", [128, cht, nh], dt.float32, kind="ExternalInput")
    bias = nc.dram_tensor("bias", [128, fin], dt.float32, kind="ExternalInput")
    if with_mm:
        W = nc.dram_tensor("W", [4, 128, fout], dt.bfloat16, kind="ExternalInput")
        asb = nc.dram_tensor("asb", [128, fout], dt.float32, kind="ExternalInput")
        adb = nc.dram_tensor("adb", [128, fout], dt.float32, kind="ExternalInput")
        ident = nc.dram_tensor("ident", [128, 128], dt.bfloat16, kind="ExternalInput")
        h_o = nc.dram_tensor("h", [NPAD, fout], dt.bfloat16, kind="ExternalOutput")
        al_o = nc.dram_tensor("al", [NPAD, 2 * n_alpha_heads], dt.float32,
                              kind="ExternalOutput")
    else:
        y_o = nc.dram_tensor("y", [NPAD, fin], dt.float32, kind="ExternalOutput")

    with tile.TileContext(nc) as tc:
        with tc.tile_pool(name="pre", bufs=1) as pre, \
             tc.tile_pool(name="fpool", bufs=2) as fpool, \
             tc.tile_pool(name="work", bufs=3) as work, \
             tc.tile_pool(name="ps", bufs=2, space="PSUM") as ps, \
             tc.tile_pool(name="pst", bufs=2, space="PSUM") as pst:
            gidx_sb = pre.tile([128, cht], dt.int32)
            nc.sync.dma_start(gidx_sb[:], gidx[:])
            dloc_sb = pre.tile([128, cht], dt.float32)
            nc.sync.dma_start(dloc_sb[:], dloc[:])
            wq_sb = pre.tile([128, cht, nh], dt.float32)
            nc.sync.dma_start(wq_sb[:], wq[:])
            bias_sb = pre.tile([128, fin], dt.float32)
            nc.sync.dma_start(bias_sb[:], bias[:])
            if with_mm:
                W_sb = pre.tile([128, 4, fout], dt.bfloat16)
                for i in range(4):
                    nc.sync.dma_start(W_sb[:, i, :], W[i])
                as_sb = pre.tile([128, fout], dt.float32)
                ad_sb = pre.tile([128, fout], dt.float32)
                nc.sync.dma_start(as_sb[:], asb[:])
                nc.sync.dma_start(ad_sb[:], adb[:])
                id_sb = pre.tile([128, 128], dt.bfloat16)
                nc.sync.dma_start(id_sb[:], ident[:])

            iota_i = pre.tile([128, 128], dt.int32)
            nc.gpsimd.iota(iota_i[:], pattern=[[1, 128]], base=0, channel_multiplier=0)
            iota_f = pre.tile([128, 128], dt.float32)
            nc.vector.tensor_copy(iota_f[:], iota_i[:])

            for b in range(NBLK):
                nchunk = int(cpb[b])
                c0 = int(off[b])
                F = fpool.tile([128, nchunk, fin // 2], dt.float32, tag="F")
                for j in range(nchunk):
                    nc.gpsimd.indirect_dma_start(
                        out=F[:, j, :], out_offset=None, in_=h_ext[:],
                        in_offset=bass.IndirectOffsetOnAxis(
                            ap=gidx_sb[:, c0 + j:c0 + j + 1], axis=0))
                agg = ps.tile([128, fin], dt.float32, tag="agg")
                for j in range(nchunk):
                    c = c0 + j
                    Fb = F[:, j, :].bitcast(dt.bfloat16)
                    if with_mm:
                        match = work.tile([128, 128], dt.bfloat16, tag="match")
                        nc.vector.tensor_scalar(
                            out=match[:], in0=iota_f[:], scalar1=dloc_sb[:, c:c + 1],
                            scalar2=None, op0=mybir.AluOpType.is_equal)
                        Fw = work.tile([128, fin], dt.bfloat16, tag="Fw")
                        for h in range(4):
                            nc.vector.tensor_scalar(
                                out=Fw[:, h * 128:(h + 1) * 128],
                                in0=Fb[:, h * 128:(h + 1) * 128],
                                scalar1=wq_sb[:, c, h:h + 1], scalar2=None,
                                op0=mybir.AluOpType.mult)
                        rhs = Fw[:]
                        lhsT = match[:]
                    else:
                        mw = work.tile([128, 128], dt.bfloat16, tag="mw")
                        nc.vector.tensor_scalar(
                            out=mw[:], in0=iota_f[:], scalar1=dloc_sb[:, c:c + 1],
                            scalar2=wq_sb[:, c, 0:1], op0=mybir.AluOpType.is_equal,
                            op1=mybir.AluOpType.mult)
                        rhs = Fb
                        lhsT = mw[:]
                    nc.tensor.matmul(agg[:], lhsT=lhsT, rhs=rhs,
                                     start=(j == 0), stop=(j == nchunk - 1))

                if not with_mm:
                    y_sb = work.tile([128, fin], dt.float32, tag="ysb")
                    nc.vector.tensor_tensor(y_sb[:], agg[:], bias_sb[:], mybir.AluOpType.add)
                    nc.sync.dma_start(y_o[b * 128:(b + 1) * 128, :], y_sb[:])
                    continue

                # out1 = ELU(agg + bias)  (f32), stored bf16 for transpose
                t0 = work.tile([128, fin], dt.float32, tag="t0")
                nc.vector.tensor_tensor(t0[:], agg[:], bias_sb[:], mybir.AluOpType.add)
                m = work.tile([128, fin], dt.float32, tag="m")
                nc.vector.tensor_scalar(out=m[:], in0=t0[:], scalar1=0.0, scalar2=None,
                                        op0=mybir.AluOpType.min)
                ex = work.tile([128, fin], dt.float32, tag="ex")
                nc.scalar.activation(ex[:], m[:], mybir.ActivationFunctionType.Exp)
                r = work.tile([128, fin], dt.float32, tag="r")
                nc.vector.tensor_scalar(out=r[:], in0=t0[:], scalar1=0.0, scalar2=-1.0,
                                        op0=mybir.AluOpType.max, op1=mybir.AluOpType.add)
                o1 = work.tile([128, fin], dt.bfloat16, tag="o1")
                nc.vector.tensor_tensor(o1[:], ex[:], r[:], mybir.AluOpType.add)

                # transpose per 128-col slice -> [c, d] tiles
                pt = pst.tile([128, fin], dt.bfloat16, tag="pt")
                for h in range(4):
                    nc.tensor.transpose(pt[:, h * 128:(h + 1) * 128],
                                        o1[:, h * 128:(h + 1) * 128], id_sb[:])
                o1T = work.tile([128, fin], dt.bfloat16, tag="o1T")
                nc.vector.tensor_copy(o1T[:], pt[:])

                hp = ps.tile([128, fout], dt.float32, tag="hp")
                for i in range(4):
                    nc.tensor.matmul(hp[:], lhsT=o1T[:, i * 128:(i + 1) * 128],
                                     rhs=W_sb[:, i, :], start=(i == 0), stop=(i == 3))
                hbf = work.tile([128, fout], dt.bfloat16, tag="hbf")
                nc.scalar.activation(hbf[:], hp[:], mybir.ActivationFunctionType.Copy)
                nc.sync.dma_start(h_o[b * 128:(b + 1) * 128, :], hbf[:])

                nah = n_alpha_heads
                al = work.tile([128, 2 * nah], dt.float32, tag="al")
                tmp = work.tile([128, fout], dt.float32, tag="tmp")
                nc.vector.tensor_tensor(tmp[:], hp[:], as_sb[:], mybir.AluOpType.mult)
                nc.vector.tensor_reduce(al[:, 0:nah],
                                        tmp.rearrange("p (h c) -> p h c", h=nah),
                                        mybir.AxisListType.X, mybir.AluOpType.add)
                nc.vector.tensor_tensor(tmp[:], hp[:], ad_sb[:], mybir.AluOpType.mult)
                nc.vector.tensor_reduce(al[:, nah:2 * nah],
                                        tmp.rearrange("p (h c) -> p h c", h=nah),
                                        mybir.AxisListType.X, mybir.AluOpType.add)
                nc.sync.dma_start(al_o[b * 128:(b + 1) * 128, :], al[:])
    nc.compile()
    return nc


def _get_progs(graph):
    key = tuple(graph.cpb.tolist())
    if key not in _PROG_CACHE:
        _PROG_CACHE[key] = {
            "k1": _build_k1(),
            "k2": _build_edge(graph, 512, 512, HEADS, True),
            "k3": _build_edge(graph, 512, HID, 1, True),
            "k4": _build_edge(graph, HID, 0, 0, False),
        }
    return _PROG_CACHE[key]


def _run(nc, in_maps, phase):
    from concourse.bass_utils import run_bass_kernel_spmd
    res = run_bass_kernel_spmd(nc, in_maps, list(range(NCORES)), trace=TRACE,
                               trace_cores=list(range(NCORES)) if TRACE else None)
    if TRACE:
        LAST_EXEC_NS[phase] = res.exec_time_ns
    return res.results


# ------------------------------------------------------------------- kernel
def kernel(x, edge_index, W1, a1_src, a1_dst, b1, W2, a2_src, a2_dst, b2,
           W3, a3_src, a3_dst, b3):
    x = np.asarray(x, np.float32)
    edge_index = np.asarray(edge_index)
    graph = _Graph(edge_index)
    progs = _get_progs(graph)
    src_all, dst_all = graph.src_all, graph.dst_all

    # ---- K1: h1 = x @ W1 ----
    W1r = np.ascontiguousarray(
        np.asarray(W1, np.float32).reshape(6, 128, 512)).astype(BF16)
    a1s_bc = _bcast(np.asarray(a1_src, np.float32).reshape(-1), 512)
    a1d_bc = _bcast(np.asarray(a1_dst, np.float32).reshape(-1), 512)
    in_maps = []
    for k in range(NCORES):
        xs = np.zeros((NPAD, IN_C), np.float32)
        xs[:NPC] = x[k * NPC:(k + 1) * NPC]
        xT = np.ascontiguousarray(xs.T.reshape(6, 128, NPAD)).astype(BF16)
        in_maps.append({"xT": xT, "W": W1r, "asb": a1s_bc, "adb": a1d_bc})
    res = _run(progs["k1"], in_maps, "k1")
    h1 = np.ascontiguousarray(
        np.concatenate([r["h"][:NPC] for r in res], axis=0))        # bf16 [N,512]
    al1 = np.concatenate([r["al"][:NPC] for r in res], axis=0)      # f32 [N,8]

    ident = np.eye(128, dtype=BF16)

    # ---- K2 ----
    wp1 = _edge_weights(al1, src_all, dst_all, HEADS)
    wq1 = graph.wq(wp1, HEADS)
    W2r = np.ascontiguousarray(
        np.asarray(W2, np.float32).reshape(4, 128, 512)).astype(BF16)
    a2s_bc = _bcast(np.asarray(a2_src, np.float32).reshape(-1), 512)
    a2d_bc = _bcast(np.asarray(a2_dst, np.float32).reshape(-1), 512)
    b1_bc = _bcast(np.asarray(b1, np.float32), 512)
    in_maps = [{"h_ext": h1.view(np.float32), "gidx": graph.gidx[k], "dloc": graph.dloc[k],
                "wq": wq1[k], "bias": b1_bc, "W": W2r, "asb": a2s_bc,
                "adb": a2d_bc, "ident": ident} for k in range(NCORES)]
    res = _run(progs["k2"], in_maps, "k2")
    h2 = np.ascontiguousarray(np.concatenate([r["h"][:NPC] for r in res], axis=0))
    al2 = np.concatenate([r["al"][:NPC] for r in res], axis=0)

    # ---- K3 ----
    wp2 = _edge_weights(al2, src_all, dst_all, HEADS)
    wq2 = graph.wq(wp2, HEADS)
    W3r = np.ascontiguousarray(
        np.asarray(W3, np.float32).reshape(4, 128, HID)).astype(BF16)
    a3s_bc = _bcast(np.asarray(a3_src, np.float32).reshape(-1), HID)
    a3d_bc = _bcast(np.asarray(a3_dst, np.float32).reshape(-1), HID)
    b2_bc = _bcast(np.asarray(b2, np.float32), 512)
    in_maps = [{"h_ext": h2.view(np.float32), "gidx": graph.gidx[k], "dloc": graph.dloc[k],
                "wq": wq2[k], "bias": b2_bc, "W": W3r, "asb": a3s_bc,
                "adb": a3d_bc, "ident": ident} for k in range(NCORES)]
    res = _run(progs["k3"], in_maps, "k3")
    h3 = np.ascontiguousarray(
        np.concatenate([r["h"][:NPC] for r in res], axis=0))        # [N,128] bf16
    al3 = np.concatenate([r["al"][:NPC] for r in res], axis=0)      # [N,2]

    # ---- K4 ----
    wp3 = _edge_weights(al3, src_all, dst_all, 1)
    wq3 = graph.wq(wp3, 1)
    b3_bc = _bcast(np.asarray(b3, np.float32), HID)
    in_maps = [{"h_ext": h3.view(np.float32), "gidx": graph.gidx[k], "dloc": graph.dloc[k],
                "wq": wq3[k], "bias": b3_bc} for k in range(NCORES)]
    res = _run(progs["k4"], in_maps, "k4")
    y = np.concatenate([r["y"][:NPC] for r in res], axis=0)
    return y.astype(np.float32)
